# revision 24
# baseline (speedup 1.0000x reference)
"""AudioAttention forward on 8 Trainium2 NeuronCores (Bass/Tile), v4.

Reference computation (eval-mode AudioAttention):
    z      = mean_pool(Z_img)                    # [B, C]
    z_img, query = z[:, :C-A], z[:, C-A:]
    snd    = Z_snd[pad_idx]                      # [G, S, C] ragged gather
    value, key = snd[..., :C-A], snd[..., C-A:]
    scores = query @ key^T  (per group), masked softmax over S
    M_snd  = attn @ value                        # [G, B, C-A]
    M_img  = broadcast(z_img)                    # [G, B, C-A]

Device strategy (per core, SPMD identical program):
  * Groups sorted by size, dealt round-robin into slots of 8 (one group per
    core per slot); slot capacity = max size in the octet.  Tokens are
    packed CONTIGUOUSLY (no chunk padding) -> ~28% less DMA than the
    128-padded layout.
  * Keys packed [128, T/2]: even chunks' 64 key features on partitions
    0-63, odd on 64-127 -> full-width DMA and ONE [128,128] weight load
    per chunk PAIR.  One N=32 matmul computes both chunks' scores
    against two half-zeroed query copies (the zero half cancels the other
    chunk; lhsT at partition base 64 mis-executes on HW, probed, so
    everything stays at base 0).  exp on ACT, 8 chunks per instruction.
    No mask and no softmax shift: queries are means over 256 pixels so
    |score| < ~3 and exp() stays inside f16 range; padded tokens have
    zero value/denominator rows so their weight cancels exactly.
  * Weighted sums: 8 groups live in one PSUM bank (16 rows each).  For
    each (chunk, 32-row group-pair) a single M=32, K=128 matmul
    m[32p:32p+32] += at2_block.T @ vals_chunk accumulates BOTH groups of
    the pair; at2_block is exp(scores) * 0/1 masks (host-built) built on
    DVE with strided/broadcast APs, so each group only credits its own
    tokens and the 450-wide value stream is paid once per chunk.
  * vals column 448 carries 1.0 for valid tokens -> the same matmul
    accumulates the softmax denominator; column 449 pads to even width.
  * Normalize: one reciprocal + one [128,448] scale per 8-group bank
    (alternating DVE/ACT), f16 out, full-width output DMA.
  * DMA: all inputs stream on the Sync HWDGE ring (qT, keys, vals pieces);
    masks ride the Scalar ring so they don't delay the value stream;
    outputs go out on Scalar between exp/evac work.
"""

import sys

if "/opt/trn_rl_repo" not in sys.path:
    sys.path.insert(0, "/opt/trn_rl_repo")

import numpy as np

N_CORES = 8
CHUNK = 128
VW = 450          # value row width: 448 features + denom + even-pad
GEXP = 8          # chunks per exp batch (even: chunk pairs share weights)
LANES = 8         # groups per PSUM m-bank (16 rows each)
WARMUP = 9        # PE warmup matmuls, N=512 (HAM un-throttle)

LAST_RESULTS = None  # BassKernelResults of the most recent run (for test harness)


def _plan(caps):
    """Static schedule shared by all cores. caps: per-slot token capacities."""
    caps = [int(c) for c in caps]
    offs = np.concatenate([[0], np.cumsum(caps)[:-1]]).astype(np.int64)
    t_used = int(offs[-1] + caps[-1]) if len(caps) else 0
    n_chunks = -(-t_used // CHUNK)
    npair = -(-n_chunks // 2)
    # pieces[k] = list of (group slot j, pa, pb) token sub-ranges of chunk k
    pieces = []
    for k in range(n_chunks):
        t0, t1 = k * CHUNK, min((k + 1) * CHUNK, t_used)
        ps = []
        for j, (o, c) in enumerate(zip(offs, caps)):
            if c == 0:
                continue
            a, b = max(t0, int(o)), min(t1, int(o) + c)
            if b > a:
                ps.append((j, a - t0, b - t0))
        pieces.append(ps)
    return offs, t_used, n_chunks, npair, pieces


def _batches(n_chunks, pieces):
    """Per exp-batch block layout.

    A block is one (chunk, group-pair): 32 at2 columns = [side0 | side1],
    one M=32 matmul.  Returns per batch: (k0, nb, blocks, w) where blocks
    is a list of (k, evac e, pair p, side_pieces) with side_pieces a dict
    side -> (pa, pb); block i occupies at2 cols [i*32, i*32+32).  The
    first block of chunk k is block index (k - k0); extra pairs of a chunk
    get appended after nb.
    """
    out = []
    n_batches = -(-n_chunks // GEXP)
    for b in range(n_batches):
        k0 = b * GEXP
        nb = min(GEXP, n_chunks - k0)
        base, extra = [], []
        for x in range(nb):
            k = k0 + x
            per_pair = {}
            order = []
            for j, pa, pb in pieces[k]:
                key = (j // LANES, (j % LANES) // 2)
                if key not in per_pair:
                    per_pair[key] = {}
                    order.append(key)
                per_pair[key][j % 2] = (pa, pb)
            base.append((k, *order[0], per_pair[order[0]]))
            for key in order[1:]:
                extra.append((k, *key, per_pair[key]))
        blocks = base + extra
        out.append((k0, nb, blocks, len(blocks)))
    return out


def _build_program(caps, gpc, ca):
    from concourse import bacc, mybir
    from concourse.tile import TileContext

    offs, t_used, n_chunks, npair, pieces = _plan(caps)
    binfo = _batches(n_chunks, pieces)
    nevac = -(-gpc // LANES)
    mask_w = sum(w for (_, _, _, w) in binfo) * 32

    nc = bacc.Bacc(None, target_bir_lowering=False, debug=False)
    f32 = mybir.dt.float32
    f16 = mybir.dt.float16
    f8 = mybir.dt.float8e3
    bf16 = mybir.dt.bfloat16
    EXP = mybir.ActivationFunctionType.Exp
    COPY = mybir.ActivationFunctionType.Copy

    keys_d = nc.dram_tensor("keys2", [128, npair * CHUNK], f8, kind="ExternalInput")
    vals_d = nc.dram_tensor("vals", [128, n_chunks * VW], f8, kind="ExternalInput")
    qt_d = nc.dram_tensor("qT", [128, 32], f16, kind="ExternalInput")
    mask_d = nc.dram_tensor("masks", [128, mask_w], f16, kind="ExternalInput")
    out_d = nc.dram_tensor("out", [128, nevac * ca], f16, kind="ExternalOutput")

    # vals DMA pieces (in chunks): small head so compute starts early
    vp, rem = [], n_chunks
    for want in (4, 6, 8, 8, 8, 2, 2):
        take = min(want, rem)
        if take:
            vp.append(take)
        rem -= take
    while rem > 0:
        vp.append(min(8, rem))
        rem -= min(8, rem)

    with TileContext(nc) as tc:
        with (
            tc.tile_pool(name="resid", bufs=1) as rpool,
            tc.tile_pool(name="ate", bufs=3) as epool,
            tc.tile_pool(name="attn", bufs=max(len(binfo), 2)) as apool,
            tc.tile_pool(name="recp", bufs=4) as recpool,
            tc.tile_pool(name="scps", bufs=3, space="PSUM") as scpsum,
            tc.tile_pool(name="mps", bufs=3, space="PSUM") as mpsum,
            tc.tile_pool(name="wps", bufs=1, space="PSUM") as wpsum,
        ):
            qt = rpool.tile([128, 32], f16)
            masks = rpool.tile([128, mask_w], f16)
            ktile = rpool.tile([128, npair * CHUNK], f8)
            vtile = rpool.tile([128, n_chunks * VW], f8)
            obuf = rpool.tile([128, nevac * ca], f16)

            # --- input DMAs: Sync ring carries the latency-critical stream;
            # masks ride the Scalar ring concurrently.  Keys split in two so
            # the first score batches unlock before the whole array lands.
            # Latency-critical small transfers ride the otherwise-empty
            # Scalar ring so their completion receipts aren't queued behind
            # the value stream; bulk data streams on Sync.
            ksplit = min(10 * CHUNK, npair * CHUNK)
            nc.scalar.dma_start(out=qt[:], in_=qt_d[:])
            nc.scalar.dma_start(out=ktile[:, 0:ksplit], in_=keys_d[:, 0:ksplit])
            nc.scalar.dma_start(out=masks[:], in_=mask_d[:])
            if ksplit < npair * CHUNK:
                nc.sync.dma_start(out=ktile[:, ksplit:], in_=keys_d[:, ksplit:])
            a = 0
            for w in vp:
                nc.sync.dma_start(
                    out=vtile[:, a * VW : (a + w) * VW],
                    in_=vals_d[:, a * VW : (a + w) * VW],
                )
                a += w

            # --- PE warmup (HAM un-throttle) while DMAs land ---
            warm = rpool.tile([128, 512], bf16)
            nc.vector.memset(warm[:], 0.0)
            wps = wpsum.tile([128, 512], f32)
            for _ in range(WARMUP):
                nc.tensor.matmul(wps[:], warm[:, :128], warm[:], start=True, stop=True)

            # --- scores -> exp -> masked attn blocks, interleaved with the
            # previous batch's weighted-sum matmuls so the PE stream has
            # score work to fill value-DMA wait gaps (keeps HAM warm).
            at_tiles = {}
            moff = 0

            def emit_scores_batch(b):
                nonlocal moff
                k0, nb, blocks, w = binfo[b]
                sc = scpsum.tile([128, nb * 16], f32, name=f"sc{b}", tag="sc")
                x = 0
                while x < nb:
                    k = k0 + x
                    n = 32 if x + 1 < nb else 16
                    nc.tensor.matmul(
                        sc[:, x * 16 : x * 16 + n],
                        ktile[:, (k // 2) * CHUNK : (k // 2 + 1) * CHUNK],
                        qt[:, (k % 2) * 16 : (k % 2) * 16 + n],
                        start=True,
                        stop=True,
                    )
                    x += n // 16
                ate = epool.tile([128, nb * 16], f16, name=f"ate{b}", tag="e")
                nc.scalar.activation(ate[:], sc[:], EXP)

                at2 = apool.tile([128, w * 32], f16, name=f"at{b}", tag="a")
                mk = masks[:, moff : moff + w * 32]
                # base blocks (one per chunk): two strided muls cover all
                # even sides then all odd sides in one instruction each.
                a2v = at2[:].rearrange("p (s t c) -> p t s c", t=2, c=16)
                mkv = mk.rearrange("p (s t c) -> p t s c", t=2, c=16)
                for side in (0, 1):
                    nc.vector.tensor_mul(
                        a2v[:, side, 0:nb, :],
                        ate[:],
                        mkv[:, side, 0:nb, :],
                    )
                # extra blocks: broadcast the chunk's exp over both sides
                for i in range(nb, w):
                    k = blocks[i][0]
                    xx = k - k0
                    nc.vector.tensor_mul(
                        at2[:, i * 32 : (i + 1) * 32].rearrange(
                            "p (t c) -> p t c", t=2
                        ),
                        ate[:, xx * 16 : (xx + 1) * 16]
                        .unsqueeze(1)
                        .broadcast_to([128, 2, 16]),
                        mk[:, i * 32 : (i + 1) * 32].rearrange(
                            "p (t c) -> p t c", t=2
                        ),
                    )
                at_tiles[b] = at2
                moff += w * 32

            # --- weighted sums: one M=32 matmul per (chunk, group-pair) ---
            bank_n = [0] * nevac
            for (k0, nb, blocks, w) in binfo:
                for (k, e, p, sides) in blocks:
                    bank_n[e] += 1
            m_tiles = {}
            bank_seen = [0] * nevac
            pair_started = set()
            evac_order = []

            def evac(e):
                m = m_tiles.pop(e)
                rec = recpool.tile([128, 1], f32, name=f"r{e}", tag="r")
                nc.vector.reciprocal(rec[:], m[:, ca : ca + 1])
                dst = obuf[:, e * ca : (e + 1) * ca]
                # ACT during the m-phase (keeps DVE free for at2 mask-muls);
                # the second-to-last bank goes to DVE so the final two
                # evacuations run on different engines concurrently.
                if e == nevac - 2:
                    nc.vector.tensor_scalar_mul(dst, m[:, 0:ca], rec[:])
                else:
                    nc.scalar.activation(dst, m[:, 0:ca], COPY, scale=rec[:])
                evac_order.append(e)
                nc.scalar.dma_start(
                    out=out_d[:, e * ca : (e + 1) * ca],
                    in_=obuf[:, e * ca : (e + 1) * ca],
                )

            def emit_m_batch(b):
                k0, nb, blocks, w = binfo[b]
                # process blocks in chunk order: base block of chunk x sits at
                # index x, extras of the batch afterwards; order by (k, index)
                order = sorted(range(w), key=lambda i: (blocks[i][0], i))
                for i in order:
                    k, e, p, sides = blocks[i]
                    if e not in m_tiles:
                        m_tiles[e] = mpsum.tile([128, VW], f32, name=f"m{e}", tag="m")
                    m = m_tiles[e]
                    start = (e, p) not in pair_started
                    pair_started.add((e, p))
                    bank_seen[e] += 1
                    stop = bank_seen[e] == bank_n[e]
                    nc.tensor.matmul(
                        m[p * 32 : p * 32 + 32, :],
                        at_tiles[b][:, i * 32 : (i + 1) * 32],
                        vtile[:, k * VW : (k + 1) * VW],
                        start=start,
                        stop=stop,
                        tile_position=(0, p * 32),
                        skip_group_check=True,
                    )
                    if stop:
                        evac(e)

            nbat = len(binfo)
            for b in range(min(2, nbat)):
                emit_scores_batch(b)
            for b in range(nbat):
                if b + 2 < nbat:
                    emit_scores_batch(b + 2)
                if b:
                    # dependency-free fillers: absorb value-DMA receipt gaps
                    # so the PE activity monitor never re-throttles mid-phase
                    for _ in range(2):
                        nc.tensor.matmul(
                            wps[:], warm[:, :128], warm[:], start=True, stop=True
                        )
                emit_m_batch(b)
            assert not m_tiles, m_tiles

    nc.finalize()
    return nc


def _build_masks(n_chunks, pieces, binfo):
    cols = []
    for (k0, nb, blocks, w) in binfo:
        blk = np.zeros((128, w * 32), dtype=np.float16)
        for i, (k, e, p, sides) in enumerate(blocks):
            for side, (pa, pb) in sides.items():
                blk[pa:pb, i * 32 + side * 16 : i * 32 + (side + 1) * 16] = 1.0
        cols.append(blk)
    return np.concatenate(cols, axis=1) if cols else np.zeros((128, 0), np.float16)


def kernel(Z_img, Z_snd, pad_idx, pad_mask, attn_dims):
    global LAST_RESULTS
    import os

    from concourse.bass_utils import run_bass_kernel_spmd

    Z_img = np.asarray(Z_img, dtype=np.float32)
    Z_snd = np.asarray(Z_snd, dtype=np.float32)
    pad_idx = np.asarray(pad_idx)
    pad_mask = np.asarray(pad_mask).astype(bool)
    A = int(attn_dims)

    B, C = Z_img.shape[0], Z_img.shape[1]
    CA = C - A
    G = pad_idx.shape[0]
    assert B == 16, B
    gpc = -(-G // N_CORES)

    z = Z_img.reshape(B, C, -1).mean(axis=2)
    z_img, query = z[:, :CA], z[:, CA:]

    sizes = pad_mask.sum(axis=1).astype(np.int64)
    order = np.argsort(-sizes, kind="stable")  # group ids, size descending
    caps = np.zeros(gpc, dtype=np.int64)
    for j in range(gpc):
        octet = sizes[order[j * N_CORES : (j + 1) * N_CORES]]
        caps[j] = max(int(octet.max()) if len(octet) else 0, 1)

    offs, t_used, n_chunks, npair, pieces = _plan(caps)
    binfo = _batches(n_chunks, pieces)
    t_pad = n_chunks * CHUNK
    nevac = -(-gpc // LANES)

    # two half-zeroed query copies: col block 0 pairs with even chunks
    # (keys on partitions 0-63), block 1 with odd chunks (64-127)
    qt = np.zeros((128, 32), dtype=np.float16)
    qt[0:64, 0:16] = query.T
    qt[64:128, 16:32] = query.T
    masks = _build_masks(n_chunks, pieces, binfo)

    in_maps = []
    for c in range(N_CORES):
        kf = np.zeros((64, npair * 2 * CHUNK), dtype=np.float32)
        va = np.zeros((t_pad, VW), dtype=np.float32)
        for j in range(gpc):
            gi = j * N_CORES + c
            if gi >= G:
                continue
            g = int(order[gi])
            s = int(sizes[g])
            o = int(offs[j])
            if s == 0:
                va[o, CA] = 1.0  # fake token: weight 1, value 0 -> output 0
                continue
            idx = pad_idx[g][pad_mask[g]]
            rows = Z_snd[idx]
            kf[:, o : o + s] = rows[:, CA:].T
            va[o : o + s, 0:CA] = rows[:, :CA]
            va[o : o + s, CA] = 1.0
        k2 = (
            kf.reshape(64, npair, 2, CHUNK)
            .transpose(2, 0, 1, 3)
            .reshape(128, npair * CHUNK)
        )
        vimg = (
            va.reshape(n_chunks, CHUNK, VW)
            .transpose(1, 0, 2)
            .reshape(128, n_chunks * VW)
        )
        import ml_dtypes

        f8 = ml_dtypes.float8_e3m4
        in_maps.append(
            {
                "keys2": np.clip(k2, -15.5, 15.5).astype(f8).view(np.uint8),
                "vals": np.clip(vimg, -15.5, 15.5).astype(f8).view(np.uint8),
                "qT": qt,
                "masks": masks,
            }
        )

    nc = _build_program(caps, gpc, CA)
    trace = bool(os.environ.get("AUDIOATTN_TRACE"))
    res = run_bass_kernel_spmd(
        nc, in_maps, list(range(N_CORES)), trace=trace,
        tmpdir=os.environ.get("AUDIOATTN_TRACE_DIR") if trace else None,
    )
    LAST_RESULTS = res

    M_snd = np.empty((G, B, CA), dtype=np.float32)
    for c in range(N_CORES):
        out_c = res.results[c]["out"].astype(np.float32)
        for j in range(gpc):
            gi = j * N_CORES + c
            if gi >= G:
                continue
            g = int(order[gi])
            e, lane = j // LANES, j % LANES
            M_snd[g] = out_c[lane * 16 : lane * 16 + 16, e * CA : (e + 1) * CA]

    M_img = np.broadcast_to(z_img[None], (G, B, CA))
    return M_img, M_snd


# revision 25
# speedup vs baseline: 1.0946x; 1.0946x over previous
"""AudioAttention forward on 8 Trainium2 NeuronCores (Bass/Tile), v4.

Reference computation (eval-mode AudioAttention):
    z      = mean_pool(Z_img)                    # [B, C]
    z_img, query = z[:, :C-A], z[:, C-A:]
    snd    = Z_snd[pad_idx]                      # [G, S, C] ragged gather
    value, key = snd[..., :C-A], snd[..., C-A:]
    scores = query @ key^T  (per group), masked softmax over S
    M_snd  = attn @ value                        # [G, B, C-A]
    M_img  = broadcast(z_img)                    # [G, B, C-A]

Device strategy (per core, SPMD identical program):
  * Groups sorted by size, dealt round-robin into slots of 8 (one group per
    core per slot); slot capacity = max size in the octet.  Tokens are
    packed CONTIGUOUSLY (no chunk padding) -> ~28% less DMA than the
    128-padded layout.
  * Keys packed [128, T/2]: even chunks' 64 key features on partitions
    0-63, odd on 64-127 -> full-width DMA and ONE [128,128] weight load
    per chunk PAIR.  One N=32 matmul computes both chunks' scores
    against two half-zeroed query copies (the zero half cancels the other
    chunk; lhsT at partition base 64 mis-executes on HW, probed, so
    everything stays at base 0).  exp on ACT, 8 chunks per instruction.
    No mask and no softmax shift: queries are means over 256 pixels so
    |score| < ~3 and exp() stays inside f16 range; padded tokens have
    zero value/denominator rows so their weight cancels exactly.
  * Weighted sums: 8 groups live in one PSUM bank (16 rows each).  For
    each (chunk, 32-row group-pair) a single M=32, K=128 matmul
    m[32p:32p+32] += at2_block.T @ vals_chunk accumulates BOTH groups of
    the pair; at2_block is exp(scores) * 0/1 masks (host-built) built on
    DVE with strided/broadcast APs, so each group only credits its own
    tokens and the 450-wide value stream is paid once per chunk.
  * vals column 448 carries 1.0 for valid tokens -> the same matmul
    accumulates the softmax denominator; column 449 pads to even width.
  * Normalize: one reciprocal + one [128,448] scale per 8-group bank
    (alternating DVE/ACT), f16 out, full-width output DMA.
  * DMA: all inputs stream on the Sync HWDGE ring (qT, keys, vals pieces);
    masks ride the Scalar ring so they don't delay the value stream;
    outputs go out on Scalar between exp/evac work.
"""

import sys

if "/opt/trn_rl_repo" not in sys.path:
    sys.path.insert(0, "/opt/trn_rl_repo")

import numpy as np

N_CORES = 8
CHUNK = 128
VW = 450          # value row width: 448 features + denom + even-pad
GEXP = 8          # chunks per exp batch (even: chunk pairs share weights)
LANES = 8         # groups per PSUM m-bank (16 rows each)
WARMUP = 9        # PE warmup matmuls, N=512 (HAM un-throttle)

LAST_RESULTS = None  # BassKernelResults of the most recent run (for test harness)


def _plan(caps):
    """Static schedule shared by all cores. caps: per-slot token capacities."""
    caps = [int(c) for c in caps]
    offs = np.concatenate([[0], np.cumsum(caps)[:-1]]).astype(np.int64)
    t_used = int(offs[-1] + caps[-1]) if len(caps) else 0
    n_chunks = -(-t_used // CHUNK)
    npair = -(-n_chunks // 2)
    # pieces[k] = list of (group slot j, pa, pb) token sub-ranges of chunk k
    pieces = []
    for k in range(n_chunks):
        t0, t1 = k * CHUNK, min((k + 1) * CHUNK, t_used)
        ps = []
        for j, (o, c) in enumerate(zip(offs, caps)):
            if c == 0:
                continue
            a, b = max(t0, int(o)), min(t1, int(o) + c)
            if b > a:
                ps.append((j, a - t0, b - t0))
        pieces.append(ps)
    return offs, t_used, n_chunks, npair, pieces


def _batches(n_chunks, pieces):
    """Per exp-batch block layout.

    A block is one (chunk, group-pair): 32 at2 columns = [side0 | side1],
    one M=32 matmul.  Returns per batch: (k0, nb, blocks, w) where blocks
    is a list of (k, evac e, pair p, side_pieces) with side_pieces a dict
    side -> (pa, pb); block i occupies at2 cols [i*32, i*32+32).  The
    first block of chunk k is block index (k - k0); extra pairs of a chunk
    get appended after nb.
    """
    out = []
    n_batches = -(-n_chunks // GEXP)
    for b in range(n_batches):
        k0 = b * GEXP
        nb = min(GEXP, n_chunks - k0)
        base, extra = [], []
        for x in range(nb):
            k = k0 + x
            per_pair = {}
            order = []
            for j, pa, pb in pieces[k]:
                key = (j // LANES, (j % LANES) // 2)
                if key not in per_pair:
                    per_pair[key] = {}
                    order.append(key)
                per_pair[key][j % 2] = (pa, pb)
            base.append((k, *order[0], per_pair[order[0]]))
            for key in order[1:]:
                extra.append((k, *key, per_pair[key]))
        blocks = base + extra
        out.append((k0, nb, blocks, len(blocks)))
    return out


def _build_program(caps, gpc, ca):
    from concourse import bacc, mybir
    from concourse.tile import TileContext

    offs, t_used, n_chunks, npair, pieces = _plan(caps)
    binfo = _batches(n_chunks, pieces)
    nevac = -(-gpc // LANES)
    mask_w = sum(w for (_, _, _, w) in binfo) * 32

    nc = bacc.Bacc(None, target_bir_lowering=False, debug=False)
    f32 = mybir.dt.float32
    f16 = mybir.dt.float16
    f8 = mybir.dt.float8e3
    bf16 = mybir.dt.bfloat16
    EXP = mybir.ActivationFunctionType.Exp
    COPY = mybir.ActivationFunctionType.Copy

    keys_d = nc.dram_tensor("keys2", [128, npair * CHUNK], f8, kind="ExternalInput")
    vals_d = nc.dram_tensor("vals", [128, n_chunks * VW], f8, kind="ExternalInput")
    qt_d = nc.dram_tensor("qT", [128, 32], f16, kind="ExternalInput")
    mask_d = nc.dram_tensor("masks", [128, mask_w], f16, kind="ExternalInput")
    out_d = nc.dram_tensor("out", [128, nevac * ca], f16, kind="ExternalOutput")

    # vals DMA pieces (in chunks): small head so compute starts early
    vp, rem = [], n_chunks
    for want in (4, 6, 8, 8, 8, 2, 2):
        take = min(want, rem)
        if take:
            vp.append(take)
        rem -= take
    while rem > 0:
        vp.append(min(8, rem))
        rem -= min(8, rem)

    with TileContext(nc) as tc:
        with (
            tc.tile_pool(name="resid", bufs=1) as rpool,
            tc.tile_pool(name="ate", bufs=3) as epool,
            tc.tile_pool(name="attn", bufs=max(len(binfo), 2)) as apool,
            tc.tile_pool(name="recp", bufs=4) as recpool,
            tc.tile_pool(name="scps", bufs=3, space="PSUM") as scpsum,
            tc.tile_pool(name="mps", bufs=3, space="PSUM") as mpsum,
            tc.tile_pool(name="wps", bufs=1, space="PSUM") as wpsum,
        ):
            qt = rpool.tile([128, 32], f16)
            masks = rpool.tile([128, mask_w], f16)
            ktile = rpool.tile([128, npair * CHUNK], f8)
            vtile = rpool.tile([128, n_chunks * VW], f8)
            obuf = rpool.tile([128, nevac * ca], f16)

            # --- input DMAs: Sync ring carries the latency-critical stream;
            # masks ride the Scalar ring concurrently.  Keys split in two so
            # the first score batches unlock before the whole array lands.
            # Latency-critical small transfers ride the otherwise-empty
            # Scalar ring so their completion receipts aren't queued behind
            # the value stream; bulk data streams on Sync.
            ksplit = min(10 * CHUNK, npair * CHUNK)
            nc.scalar.dma_start(out=qt[:], in_=qt_d[:])
            nc.scalar.dma_start(out=ktile[:, 0:ksplit], in_=keys_d[:, 0:ksplit])
            nc.scalar.dma_start(out=masks[:], in_=mask_d[:])
            if ksplit < npair * CHUNK:
                nc.sync.dma_start(out=ktile[:, ksplit:], in_=keys_d[:, ksplit:])
            a = 0
            for w in vp:
                nc.sync.dma_start(
                    out=vtile[:, a * VW : (a + w) * VW],
                    in_=vals_d[:, a * VW : (a + w) * VW],
                )
                a += w

            # --- PE warmup (HAM un-throttle) while DMAs land ---
            warm = rpool.tile([128, 512], bf16)
            nc.vector.memset(warm[:], 0.0)
            wps = wpsum.tile([128, 512], f32)
            for _ in range(WARMUP):
                nc.tensor.matmul(wps[:], warm[:, :128], warm[:], start=True, stop=True)

            # --- scores -> exp -> masked attn blocks, interleaved with the
            # previous batch's weighted-sum matmuls so the PE stream has
            # score work to fill value-DMA wait gaps (keeps HAM warm).
            at_tiles = {}
            moff = 0

            def emit_scores_batch(b):
                nonlocal moff
                k0, nb, blocks, w = binfo[b]
                sc = scpsum.tile([128, nb * 16], f32, name=f"sc{b}", tag="sc")
                x = 0
                while x < nb:
                    k = k0 + x
                    n = 32 if x + 1 < nb else 16
                    nc.tensor.matmul(
                        sc[:, x * 16 : x * 16 + n],
                        ktile[:, (k // 2) * CHUNK : (k // 2 + 1) * CHUNK],
                        qt[:, (k % 2) * 16 : (k % 2) * 16 + n],
                        start=True,
                        stop=True,
                    )
                    x += n // 16
                ate = epool.tile([128, nb * 16], f16, name=f"ate{b}", tag="e")
                nc.scalar.activation(ate[:], sc[:], EXP)

                at2 = apool.tile([128, w * 32], f16, name=f"at{b}", tag="a")
                mk = masks[:, moff : moff + w * 32]
                # base blocks (one per chunk): two strided muls cover all
                # even sides then all odd sides in one instruction each.
                a2v = at2[:].rearrange("p (s t c) -> p t s c", t=2, c=16)
                mkv = mk.rearrange("p (s t c) -> p t s c", t=2, c=16)
                for side in (0, 1):
                    nc.vector.tensor_mul(
                        a2v[:, side, 0:nb, :],
                        ate[:],
                        mkv[:, side, 0:nb, :],
                    )
                # extra blocks: broadcast the chunk's exp over both sides
                for i in range(nb, w):
                    k = blocks[i][0]
                    xx = k - k0
                    nc.vector.tensor_mul(
                        at2[:, i * 32 : (i + 1) * 32].rearrange(
                            "p (t c) -> p t c", t=2
                        ),
                        ate[:, xx * 16 : (xx + 1) * 16]
                        .unsqueeze(1)
                        .broadcast_to([128, 2, 16]),
                        mk[:, i * 32 : (i + 1) * 32].rearrange(
                            "p (t c) -> p t c", t=2
                        ),
                    )
                at_tiles[b] = at2
                moff += w * 32

            # --- weighted sums: one M=32 matmul per (chunk, group-pair) ---
            bank_n = [0] * nevac
            for (k0, nb, blocks, w) in binfo:
                for (k, e, p, sides) in blocks:
                    bank_n[e] += 1
            m_tiles = {}
            bank_seen = [0] * nevac
            pair_started = set()
            evac_order = []

            def evac(e):
                m = m_tiles.pop(e)
                rec = recpool.tile([128, 1], f32, name=f"r{e}", tag="r")
                nc.vector.reciprocal(rec[:], m[:, ca : ca + 1])
                dst = obuf[:, e * ca : (e + 1) * ca]
                # ACT during the m-phase (keeps DVE free for at2 mask-muls);
                # the second-to-last bank goes to DVE so the final two
                # evacuations run on different engines concurrently.
                if e == nevac - 2:
                    nc.vector.tensor_scalar_mul(dst, m[:, 0:ca], rec[:])
                else:
                    nc.scalar.activation(dst, m[:, 0:ca], COPY, scale=rec[:])
                evac_order.append(e)
                if len(evac_order) % 2 == 0:
                    e0 = min(evac_order[-2:])
                    nc.scalar.dma_start(
                        out=out_d[:, e0 * ca : (e0 + 2) * ca],
                        in_=obuf[:, e0 * ca : (e0 + 2) * ca],
                    )

            def emit_m_batch(b):
                k0, nb, blocks, w = binfo[b]
                # process blocks in chunk order: base block of chunk x sits at
                # index x, extras of the batch afterwards; order by (k, index)
                order = sorted(range(w), key=lambda i: (blocks[i][0], i))
                for i in order:
                    k, e, p, sides = blocks[i]
                    if e not in m_tiles:
                        m_tiles[e] = mpsum.tile([128, VW], f32, name=f"m{e}", tag="m")
                    m = m_tiles[e]
                    start = (e, p) not in pair_started
                    pair_started.add((e, p))
                    bank_seen[e] += 1
                    stop = bank_seen[e] == bank_n[e]
                    nc.tensor.matmul(
                        m[p * 32 : p * 32 + 32, :],
                        at_tiles[b][:, i * 32 : (i + 1) * 32],
                        vtile[:, k * VW : (k + 1) * VW],
                        start=start,
                        stop=stop,
                        tile_position=(0, p * 32),
                        skip_group_check=True,
                    )
                    if stop:
                        evac(e)

            nbat = len(binfo)
            for b in range(min(2, nbat)):
                emit_scores_batch(b)
            for b in range(nbat):
                if b + 2 < nbat:
                    emit_scores_batch(b + 2)
                if b:
                    # dependency-free fillers: absorb value-DMA receipt gaps
                    # so the PE activity monitor never re-throttles mid-phase
                    for _ in range(2):
                        nc.tensor.matmul(
                            wps[:], warm[:, :128], warm[:], start=True, stop=True
                        )
                emit_m_batch(b)
            assert not m_tiles, m_tiles
            if len(evac_order) % 2 == 1:
                e0 = evac_order[-1]
                nc.scalar.dma_start(
                    out=out_d[:, e0 * ca : (e0 + 1) * ca],
                    in_=obuf[:, e0 * ca : (e0 + 1) * ca],
                )

    nc.finalize()
    return nc


def _build_masks(n_chunks, pieces, binfo):
    cols = []
    for (k0, nb, blocks, w) in binfo:
        blk = np.zeros((128, w * 32), dtype=np.float16)
        for i, (k, e, p, sides) in enumerate(blocks):
            for side, (pa, pb) in sides.items():
                blk[pa:pb, i * 32 + side * 16 : i * 32 + (side + 1) * 16] = 1.0
        cols.append(blk)
    return np.concatenate(cols, axis=1) if cols else np.zeros((128, 0), np.float16)


def kernel(Z_img, Z_snd, pad_idx, pad_mask, attn_dims):
    global LAST_RESULTS
    import os

    from concourse.bass_utils import run_bass_kernel_spmd

    Z_img = np.asarray(Z_img, dtype=np.float32)
    Z_snd = np.asarray(Z_snd, dtype=np.float32)
    pad_idx = np.asarray(pad_idx)
    pad_mask = np.asarray(pad_mask).astype(bool)
    A = int(attn_dims)

    B, C = Z_img.shape[0], Z_img.shape[1]
    CA = C - A
    G = pad_idx.shape[0]
    assert B == 16, B
    gpc = -(-G // N_CORES)

    z = Z_img.reshape(B, C, -1).mean(axis=2)
    z_img, query = z[:, :CA], z[:, CA:]

    sizes = pad_mask.sum(axis=1).astype(np.int64)
    order = np.argsort(-sizes, kind="stable")  # group ids, size descending
    caps = np.zeros(gpc, dtype=np.int64)
    for j in range(gpc):
        octet = sizes[order[j * N_CORES : (j + 1) * N_CORES]]
        caps[j] = max(int(octet.max()) if len(octet) else 0, 1)

    offs, t_used, n_chunks, npair, pieces = _plan(caps)
    binfo = _batches(n_chunks, pieces)
    t_pad = n_chunks * CHUNK
    nevac = -(-gpc // LANES)

    # two half-zeroed query copies: col block 0 pairs with even chunks
    # (keys on partitions 0-63), block 1 with odd chunks (64-127)
    qt = np.zeros((128, 32), dtype=np.float16)
    qt[0:64, 0:16] = query.T
    qt[64:128, 16:32] = query.T
    masks = _build_masks(n_chunks, pieces, binfo)

    in_maps = []
    for c in range(N_CORES):
        kf = np.zeros((64, npair * 2 * CHUNK), dtype=np.float32)
        va = np.zeros((t_pad, VW), dtype=np.float32)
        for j in range(gpc):
            gi = j * N_CORES + c
            if gi >= G:
                continue
            g = int(order[gi])
            s = int(sizes[g])
            o = int(offs[j])
            if s == 0:
                va[o, CA] = 1.0  # fake token: weight 1, value 0 -> output 0
                continue
            idx = pad_idx[g][pad_mask[g]]
            rows = Z_snd[idx]
            kf[:, o : o + s] = rows[:, CA:].T
            va[o : o + s, 0:CA] = rows[:, :CA]
            va[o : o + s, CA] = 1.0
        k2 = (
            kf.reshape(64, npair, 2, CHUNK)
            .transpose(2, 0, 1, 3)
            .reshape(128, npair * CHUNK)
        )
        vimg = (
            va.reshape(n_chunks, CHUNK, VW)
            .transpose(1, 0, 2)
            .reshape(128, n_chunks * VW)
        )
        import ml_dtypes

        f8 = ml_dtypes.float8_e3m4
        in_maps.append(
            {
                "keys2": np.clip(k2, -15.5, 15.5).astype(f8).view(np.uint8),
                "vals": np.clip(vimg, -15.5, 15.5).astype(f8).view(np.uint8),
                "qT": qt,
                "masks": masks,
            }
        )

    nc = _build_program(caps, gpc, CA)
    trace = bool(os.environ.get("AUDIOATTN_TRACE"))
    res = run_bass_kernel_spmd(
        nc, in_maps, list(range(N_CORES)), trace=trace,
        tmpdir=os.environ.get("AUDIOATTN_TRACE_DIR") if trace else None,
    )
    LAST_RESULTS = res

    M_snd = np.empty((G, B, CA), dtype=np.float32)
    for c in range(N_CORES):
        out_c = res.results[c]["out"].astype(np.float32)
        for j in range(gpc):
            gi = j * N_CORES + c
            if gi >= G:
                continue
            g = int(order[gi])
            e, lane = j // LANES, j % LANES
            M_snd[g] = out_c[lane * 16 : lane * 16 + 16, e * CA : (e + 1) * CA]

    M_img = np.broadcast_to(z_img[None], (G, B, CA))
    return M_img, M_snd


# revision 26
# speedup vs baseline: 1.1670x; 1.0662x over previous
"""AudioAttention forward on 8 Trainium2 NeuronCores (Bass/Tile), v4.

Reference computation (eval-mode AudioAttention):
    z      = mean_pool(Z_img)                    # [B, C]
    z_img, query = z[:, :C-A], z[:, C-A:]
    snd    = Z_snd[pad_idx]                      # [G, S, C] ragged gather
    value, key = snd[..., :C-A], snd[..., C-A:]
    scores = query @ key^T  (per group), masked softmax over S
    M_snd  = attn @ value                        # [G, B, C-A]
    M_img  = broadcast(z_img)                    # [G, B, C-A]

Device strategy (per core, SPMD identical program):
  * Groups sorted by size, dealt round-robin into slots of 8 (one group per
    core per slot); slot capacity = max size in the octet.  Tokens are
    packed CONTIGUOUSLY (no chunk padding) -> ~28% less DMA than the
    128-padded layout.
  * Keys packed [128, T/2]: even chunks' 64 key features on partitions
    0-63, odd on 64-127 -> full-width DMA and ONE [128,128] weight load
    per chunk PAIR.  One N=32 matmul computes both chunks' scores
    against two half-zeroed query copies (the zero half cancels the other
    chunk; lhsT at partition base 64 mis-executes on HW, probed, so
    everything stays at base 0).  exp on ACT, 8 chunks per instruction.
    No mask and no softmax shift: queries are means over 256 pixels so
    |score| < ~3 and exp() stays inside f16 range; padded tokens have
    zero value/denominator rows so their weight cancels exactly.
  * Weighted sums: 8 groups live in one PSUM bank (16 rows each).  For
    each (chunk, 32-row group-pair) a single M=32, K=128 matmul
    m[32p:32p+32] += at2_block.T @ vals_chunk accumulates BOTH groups of
    the pair; at2_block is exp(scores) * 0/1 masks (host-built) built on
    DVE with strided/broadcast APs, so each group only credits its own
    tokens and the 450-wide value stream is paid once per chunk.
  * vals column 448 carries 1.0 for valid tokens -> the same matmul
    accumulates the softmax denominator; column 449 pads to even width.
  * Normalize: one reciprocal + one [128,448] scale per 8-group bank
    (alternating DVE/ACT), f16 out, full-width output DMA.
  * DMA: all inputs stream on the Sync HWDGE ring (qT, keys, vals pieces);
    masks ride the Scalar ring so they don't delay the value stream;
    outputs go out on Scalar between exp/evac work.
"""

import sys

if "/opt/trn_rl_repo" not in sys.path:
    sys.path.insert(0, "/opt/trn_rl_repo")

import numpy as np

N_CORES = 8
CHUNK = 128
VW = 450          # value row width: 448 features + denom + even-pad
GEXP = 8          # chunks per exp batch (even: chunk pairs share weights)
LANES = 8         # groups per PSUM m-bank (16 rows each)
WARMUP = 9        # PE warmup matmuls, N=512 (HAM un-throttle)

LAST_RESULTS = None  # BassKernelResults of the most recent run (for test harness)


def _plan(caps):
    """Static schedule shared by all cores. caps: per-slot token capacities."""
    caps = [int(c) for c in caps]
    offs = np.concatenate([[0], np.cumsum(caps)[:-1]]).astype(np.int64)
    t_used = int(offs[-1] + caps[-1]) if len(caps) else 0
    n_chunks = -(-t_used // CHUNK)
    npair = -(-n_chunks // 2)
    # pieces[k] = list of (group slot j, pa, pb) token sub-ranges of chunk k
    pieces = []
    for k in range(n_chunks):
        t0, t1 = k * CHUNK, min((k + 1) * CHUNK, t_used)
        ps = []
        for j, (o, c) in enumerate(zip(offs, caps)):
            if c == 0:
                continue
            a, b = max(t0, int(o)), min(t1, int(o) + c)
            if b > a:
                ps.append((j, a - t0, b - t0))
        pieces.append(ps)
    return offs, t_used, n_chunks, npair, pieces


def _batches(n_chunks, pieces):
    """Per exp-batch block layout.

    A block is one (chunk, group-pair): 32 at2 columns = [side0 | side1],
    one M=32 matmul.  Returns per batch: (k0, nb, blocks, w) where blocks
    is a list of (k, evac e, pair p, side_pieces) with side_pieces a dict
    side -> (pa, pb); block i occupies at2 cols [i*32, i*32+32).  The
    first block of chunk k is block index (k - k0); extra pairs of a chunk
    get appended after nb.
    """
    out = []
    n_batches = -(-n_chunks // GEXP)
    for b in range(n_batches):
        k0 = b * GEXP
        nb = min(GEXP, n_chunks - k0)
        base, extra = [], []
        for x in range(nb):
            k = k0 + x
            per_pair = {}
            order = []
            for j, pa, pb in pieces[k]:
                key = (j // LANES, (j % LANES) // 2)
                if key not in per_pair:
                    per_pair[key] = {}
                    order.append(key)
                per_pair[key][j % 2] = (pa, pb)
            base.append((k, *order[0], per_pair[order[0]]))
            for key in order[1:]:
                extra.append((k, *key, per_pair[key]))
        blocks = base + extra
        out.append((k0, nb, blocks, len(blocks)))
    return out


def _build_program(caps, gpc, ca):
    from concourse import bacc, mybir
    from concourse.tile import TileContext

    offs, t_used, n_chunks, npair, pieces = _plan(caps)
    binfo = _batches(n_chunks, pieces)
    nevac = -(-gpc // LANES)
    mask_w = sum(w for (_, _, _, w) in binfo) * 32

    nc = bacc.Bacc(None, target_bir_lowering=False, debug=False)
    f32 = mybir.dt.float32
    f16 = mybir.dt.float16
    f8 = mybir.dt.float8e3
    bf16 = mybir.dt.bfloat16
    EXP = mybir.ActivationFunctionType.Exp
    COPY = mybir.ActivationFunctionType.Copy

    keys_d = nc.dram_tensor("keys2", [128, npair * CHUNK], f8, kind="ExternalInput")

    qt_d = nc.dram_tensor("qT", [128, 32], f16, kind="ExternalInput")
    mask_d = nc.dram_tensor("masks", [128, mask_w], f16, kind="ExternalInput")
    out_d = nc.dram_tensor("out", [128, nevac * ca], f16, kind="ExternalOutput")

    # vals DMA pieces (in chunks): small head so compute starts early,
    # small tail so the last compute burst isn't gated on a big transfer.
    # Each piece is its own DRAM tensor -> fully contiguous HBM reads.
    vp, rem = [], n_chunks
    for want in (6, 10, 10, 8, 2, 2):
        take = min(want, rem)
        if take:
            vp.append(take)
        rem -= take
    while rem > 0:
        vp.append(min(10, rem))
        rem -= min(10, rem)
    vals_ds = [
        nc.dram_tensor(f"vals{i}", [128, w * VW], f8, kind="ExternalInput")
        for i, w in enumerate(vp)
    ]

    with TileContext(nc) as tc:
        with (
            tc.tile_pool(name="resid", bufs=1) as rpool,
            tc.tile_pool(name="ate", bufs=3) as epool,
            tc.tile_pool(name="attn", bufs=max(len(binfo), 2)) as apool,
            tc.tile_pool(name="recp", bufs=4) as recpool,
            tc.tile_pool(name="scps", bufs=3, space="PSUM") as scpsum,
            tc.tile_pool(name="mps", bufs=3, space="PSUM") as mpsum,
            tc.tile_pool(name="wps", bufs=1, space="PSUM") as wpsum,
        ):
            qt = rpool.tile([128, 32], f16)
            masks = rpool.tile([128, mask_w], f16)
            ktile = rpool.tile([128, npair * CHUNK], f8)
            vtile = rpool.tile([128, n_chunks * VW], f8)
            obuf = rpool.tile([128, nevac * ca], f16)

            # --- input DMAs: Sync ring carries the latency-critical stream;
            # masks ride the Scalar ring concurrently.  Keys split in two so
            # the first score batches unlock before the whole array lands.
            # Latency-critical small transfers ride the otherwise-empty
            # Scalar ring so their completion receipts aren't queued behind
            # the value stream; bulk data streams on Sync.
            ksplit = min(10 * CHUNK, npair * CHUNK)
            nc.scalar.dma_start(out=qt[:], in_=qt_d[:])
            nc.scalar.dma_start(out=ktile[:, 0:ksplit], in_=keys_d[:, 0:ksplit])
            nc.scalar.dma_start(out=masks[:], in_=mask_d[:])
            if ksplit < npair * CHUNK:
                nc.sync.dma_start(out=ktile[:, ksplit:], in_=keys_d[:, ksplit:])
            a = 0
            for i, w in enumerate(vp):
                nc.sync.dma_start(
                    out=vtile[:, a * VW : (a + w) * VW],
                    in_=vals_ds[i][:],
                )
                a += w

            # --- PE warmup (HAM un-throttle) while DMAs land ---
            warm = rpool.tile([128, 512], bf16)
            nc.vector.memset(warm[:], 0.0)
            wps = wpsum.tile([128, 512], f32)
            for _ in range(WARMUP):
                nc.tensor.matmul(wps[:], warm[:, :128], warm[:], start=True, stop=True)

            # --- scores -> exp -> masked attn blocks, interleaved with the
            # previous batch's weighted-sum matmuls so the PE stream has
            # score work to fill value-DMA wait gaps (keeps HAM warm).
            at_tiles = {}
            moff = 0

            def emit_scores_batch(b):
                nonlocal moff
                k0, nb, blocks, w = binfo[b]
                sc = scpsum.tile([128, nb * 16], f32, name=f"sc{b}", tag="sc")
                x = 0
                while x < nb:
                    k = k0 + x
                    n = 32 if x + 1 < nb else 16
                    nc.tensor.matmul(
                        sc[:, x * 16 : x * 16 + n],
                        ktile[:, (k // 2) * CHUNK : (k // 2 + 1) * CHUNK],
                        qt[:, (k % 2) * 16 : (k % 2) * 16 + n],
                        start=True,
                        stop=True,
                    )
                    x += n // 16
                ate = epool.tile([128, nb * 16], f16, name=f"ate{b}", tag="e")
                nc.scalar.activation(ate[:], sc[:], EXP)

                at2 = apool.tile([128, w * 32], f16, name=f"at{b}", tag="a")
                mk = masks[:, moff : moff + w * 32]
                # base blocks (one per chunk): two strided muls cover all
                # even sides then all odd sides in one instruction each.
                a2v = at2[:].rearrange("p (s t c) -> p t s c", t=2, c=16)
                mkv = mk.rearrange("p (s t c) -> p t s c", t=2, c=16)
                for side in (0, 1):
                    nc.vector.tensor_mul(
                        a2v[:, side, 0:nb, :],
                        ate[:],
                        mkv[:, side, 0:nb, :],
                    )
                # extra blocks: broadcast the chunk's exp over both sides
                for i in range(nb, w):
                    k = blocks[i][0]
                    xx = k - k0
                    nc.vector.tensor_mul(
                        at2[:, i * 32 : (i + 1) * 32].rearrange(
                            "p (t c) -> p t c", t=2
                        ),
                        ate[:, xx * 16 : (xx + 1) * 16]
                        .unsqueeze(1)
                        .broadcast_to([128, 2, 16]),
                        mk[:, i * 32 : (i + 1) * 32].rearrange(
                            "p (t c) -> p t c", t=2
                        ),
                    )
                at_tiles[b] = at2
                moff += w * 32

            # --- weighted sums: one M=32 matmul per (chunk, group-pair) ---
            bank_n = [0] * nevac
            for (k0, nb, blocks, w) in binfo:
                for (k, e, p, sides) in blocks:
                    bank_n[e] += 1
            m_tiles = {}
            bank_seen = [0] * nevac
            pair_started = set()
            evac_order = []

            def evac(e):
                m = m_tiles.pop(e)
                rec = recpool.tile([128, 1], f32, name=f"r{e}", tag="r")
                nc.vector.reciprocal(rec[:], m[:, ca : ca + 1])
                dst = obuf[:, e * ca : (e + 1) * ca]
                # ACT during the m-phase (keeps DVE free for at2 mask-muls);
                # the second-to-last bank goes to DVE so the final two
                # evacuations run on different engines concurrently.
                if e == nevac - 2:
                    nc.vector.tensor_scalar_mul(dst, m[:, 0:ca], rec[:])
                else:
                    nc.scalar.activation(dst, m[:, 0:ca], COPY, scale=rec[:])
                evac_order.append(e)
                if len(evac_order) % 2 == 0:
                    e0 = min(evac_order[-2:])
                    nc.scalar.dma_start(
                        out=out_d[:, e0 * ca : (e0 + 2) * ca],
                        in_=obuf[:, e0 * ca : (e0 + 2) * ca],
                    )

            def emit_m_batch(b):
                k0, nb, blocks, w = binfo[b]
                # process blocks in chunk order: base block of chunk x sits at
                # index x, extras of the batch afterwards; order by (k, index)
                order = sorted(range(w), key=lambda i: (blocks[i][0], i))
                for i in order:
                    k, e, p, sides = blocks[i]
                    if e not in m_tiles:
                        m_tiles[e] = mpsum.tile([128, VW], f32, name=f"m{e}", tag="m")
                    m = m_tiles[e]
                    start = (e, p) not in pair_started
                    pair_started.add((e, p))
                    bank_seen[e] += 1
                    stop = bank_seen[e] == bank_n[e]
                    nc.tensor.matmul(
                        m[p * 32 : p * 32 + 32, :],
                        at_tiles[b][:, i * 32 : (i + 1) * 32],
                        vtile[:, k * VW : (k + 1) * VW],
                        start=start,
                        stop=stop,
                        tile_position=(0, p * 32),
                        skip_group_check=True,
                    )
                    if stop:
                        evac(e)

            nbat = len(binfo)
            for b in range(min(2, nbat)):
                emit_scores_batch(b)
            for b in range(nbat):
                if b + 2 < nbat:
                    emit_scores_batch(b + 2)
                if b:
                    # dependency-free fillers: absorb value-DMA receipt gaps
                    # so the PE activity monitor never re-throttles mid-phase
                    for _ in range(2):
                        nc.tensor.matmul(
                            wps[:], warm[:, :128], warm[:], start=True, stop=True
                        )
                emit_m_batch(b)
            assert not m_tiles, m_tiles
            if len(evac_order) % 2 == 1:
                e0 = evac_order[-1]
                nc.scalar.dma_start(
                    out=out_d[:, e0 * ca : (e0 + 1) * ca],
                    in_=obuf[:, e0 * ca : (e0 + 1) * ca],
                )

    nc.finalize()
    return nc


def _build_masks(n_chunks, pieces, binfo):
    cols = []
    for (k0, nb, blocks, w) in binfo:
        blk = np.zeros((128, w * 32), dtype=np.float16)
        for i, (k, e, p, sides) in enumerate(blocks):
            for side, (pa, pb) in sides.items():
                blk[pa:pb, i * 32 + side * 16 : i * 32 + (side + 1) * 16] = 1.0
        cols.append(blk)
    return np.concatenate(cols, axis=1) if cols else np.zeros((128, 0), np.float16)


def kernel(Z_img, Z_snd, pad_idx, pad_mask, attn_dims):
    global LAST_RESULTS
    import os

    from concourse.bass_utils import run_bass_kernel_spmd

    Z_img = np.asarray(Z_img, dtype=np.float32)
    Z_snd = np.asarray(Z_snd, dtype=np.float32)
    pad_idx = np.asarray(pad_idx)
    pad_mask = np.asarray(pad_mask).astype(bool)
    A = int(attn_dims)

    B, C = Z_img.shape[0], Z_img.shape[1]
    CA = C - A
    G = pad_idx.shape[0]
    assert B == 16, B
    gpc = -(-G // N_CORES)

    z = Z_img.reshape(B, C, -1).mean(axis=2)
    z_img, query = z[:, :CA], z[:, CA:]

    sizes = pad_mask.sum(axis=1).astype(np.int64)
    order = np.argsort(-sizes, kind="stable")  # group ids, size descending
    caps = np.zeros(gpc, dtype=np.int64)
    for j in range(gpc):
        octet = sizes[order[j * N_CORES : (j + 1) * N_CORES]]
        caps[j] = max(int(octet.max()) if len(octet) else 0, 1)

    offs, t_used, n_chunks, npair, pieces = _plan(caps)
    binfo = _batches(n_chunks, pieces)
    t_pad = n_chunks * CHUNK
    nevac = -(-gpc // LANES)

    # two half-zeroed query copies: col block 0 pairs with even chunks
    # (keys on partitions 0-63), block 1 with odd chunks (64-127)
    qt = np.zeros((128, 32), dtype=np.float16)
    qt[0:64, 0:16] = query.T
    qt[64:128, 16:32] = query.T
    masks = _build_masks(n_chunks, pieces, binfo)

    in_maps = []
    for c in range(N_CORES):
        kf = np.zeros((64, npair * 2 * CHUNK), dtype=np.float32)
        va = np.zeros((t_pad, VW), dtype=np.float32)
        for j in range(gpc):
            gi = j * N_CORES + c
            if gi >= G:
                continue
            g = int(order[gi])
            s = int(sizes[g])
            o = int(offs[j])
            if s == 0:
                va[o, CA] = 1.0  # fake token: weight 1, value 0 -> output 0
                continue
            idx = pad_idx[g][pad_mask[g]]
            rows = Z_snd[idx]
            kf[:, o : o + s] = rows[:, CA:].T
            va[o : o + s, 0:CA] = rows[:, :CA]
            va[o : o + s, CA] = 1.0
        k2 = (
            kf.reshape(64, npair, 2, CHUNK)
            .transpose(2, 0, 1, 3)
            .reshape(128, npair * CHUNK)
        )
        vimg = (
            va.reshape(n_chunks, CHUNK, VW)
            .transpose(1, 0, 2)
            .reshape(128, n_chunks * VW)
        )
        import ml_dtypes

        f8 = ml_dtypes.float8_e3m4
        vp, rem = [], n_chunks
        for want in (6, 10, 10, 8, 2, 2):
            take = min(want, rem)
            if take:
                vp.append(take)
            rem -= take
        while rem > 0:
            vp.append(min(10, rem))
            rem -= min(10, rem)
        v8 = np.clip(vimg, -15.5, 15.5).astype(f8).view(np.uint8)
        im = {
            "keys2": np.clip(k2, -15.5, 15.5).astype(f8).view(np.uint8),
            "qT": qt,
            "masks": masks,
        }
        a = 0
        for i, w in enumerate(vp):
            im[f"vals{i}"] = np.ascontiguousarray(v8[:, a * VW : (a + w) * VW])
            a += w
        in_maps.append(im)

    nc = _build_program(caps, gpc, CA)
    trace = bool(os.environ.get("AUDIOATTN_TRACE"))
    res = run_bass_kernel_spmd(
        nc, in_maps, list(range(N_CORES)), trace=trace,
        tmpdir=os.environ.get("AUDIOATTN_TRACE_DIR") if trace else None,
    )
    LAST_RESULTS = res

    M_snd = np.empty((G, B, CA), dtype=np.float32)
    for c in range(N_CORES):
        out_c = res.results[c]["out"].astype(np.float32)
        for j in range(gpc):
            gi = j * N_CORES + c
            if gi >= G:
                continue
            g = int(order[gi])
            e, lane = j // LANES, j % LANES
            M_snd[g] = out_c[lane * 16 : lane * 16 + 16, e * CA : (e + 1) * CA]

    M_img = np.broadcast_to(z_img[None], (G, B, CA))
    return M_img, M_snd
